# revision 4
# baseline (speedup 1.0000x reference)
"""Trainium2 Bass kernel for nn_CrossAttentionBlock_78881369358733.

The reference block's attention is degenerate: every query attends to a
single broadcast context token, so softmax over N identical scores is
exactly uniform and the attention output equals `v` for every position.
The whole module collapses to

    ctx   = param_tokens @ Wparam + bparam          # [B, C]
    v     = layernorm(ctx) @ Wkv[:, C:]             # [B, C]
    delta = v @ Wout + bout                         # [B, C]
    out   = img_tokens + delta[:, None, :]          # [B, N, C]

(q, Wq, img layernorm params, and the k-half of Wkv are dead.)

Sharding: pure data parallel over B — core b handles batch b. Each core
computes its own tiny delta vector on-device (PE matmuls + DVE/ACT ops)
and streams img tiles through a broadcast add.
"""

import sys

if "/opt/trn_rl_repo" not in sys.path:
    sys.path.append("/opt/trn_rl_repo")

import numpy as np

B, N, C = 8, 1024, 256
PARAM_DIM = 16
EPS = 1e-5
P = 128
NT = N // P  # img tiles per core
NCORES = 8

_BUILT = None


def _patch_tile_tail(tile_mod):
    """The stock TileContext tail emits a drain with one sem-wait per live
    proc (rejected by this walrus: >2 sync waits per TPB_CTRL) followed by
    an EVSEM barrier + sem reset that faults the exec unit on this runtime.
    A single drain is sufficient for one-shot NEFF execution: semaphores
    are re-initialized by each nrt_execute."""
    from bass_rust import ScopedClock

    def _drain_only(self, tick_clock, wait_clock):
        drain_inst = self.nc.sync.drain()
        wait_clock.add_sem_waits(
            drain_inst.ins, ScopedClock({None: tick_clock.global_clock})
        )
        popped = self.nc._tile_sem_poison_stack.pop()
        assert popped is self._sem_poison

    tile_mod.TileContext._drain_and_barrier = _drain_only


def _build():
    import concourse.bacc as bacc
    import concourse.tile as tile
    from concourse import mybir

    _patch_tile_tail(tile)

    f32 = mybir.dt.float32
    AF = mybir.ActivationFunctionType
    ALU = mybir.AluOpType

    nc = bacc.Bacc("TRN2", target_bir_lowering=False, debug=False)

    img = nc.dram_tensor("img", [N, C], f32, kind="ExternalInput")
    param = nc.dram_tensor("param", [PARAM_DIM], f32, kind="ExternalInput")
    wparam = nc.dram_tensor("wparam", [PARAM_DIM, C], f32, kind="ExternalInput")
    bparam = nc.dram_tensor("bparam", [C], f32, kind="ExternalInput")
    gln = nc.dram_tensor("gln", [C], f32, kind="ExternalInput")
    bln = nc.dram_tensor("bln", [C], f32, kind="ExternalInput")
    wv = nc.dram_tensor("wv", [C, C], f32, kind="ExternalInput")
    wout = nc.dram_tensor("wout", [C, C], f32, kind="ExternalInput")
    bout = nc.dram_tensor("bout", [C], f32, kind="ExternalInput")
    out = nc.dram_tensor("out", [N, C], f32, kind="ExternalOutput")

    with tile.TileContext(nc) as tc:
        with (
            tc.tile_pool(name="w", bufs=1) as wp,
            tc.tile_pool(name="io", bufs=NT) as iop,
            tc.tile_pool(name="ps", bufs=1, space="PSUM") as pp,
        ):
            # ---- small/weight loads (emitted first: they gate the chain) ----
            pT = wp.tile([PARAM_DIM, 1], f32)
            nc.sync.dma_start(pT[:], param.ap().rearrange("(k o) -> k o", o=1))
            wparam_sb = wp.tile([PARAM_DIM, C], f32)
            nc.sync.dma_start(wparam_sb[:], wparam.ap())
            bparamT = wp.tile([P, 2], f32)
            nc.sync.dma_start(bparamT[:], bparam.ap().rearrange("(j p) -> p j", p=P))
            gT = wp.tile([P, 2], f32)
            nc.sync.dma_start(gT[:], gln.ap().rearrange("(j p) -> p j", p=P))
            bT = wp.tile([P, 2], f32)
            nc.sync.dma_start(bT[:], bln.ap().rearrange("(j p) -> p j", p=P))
            wv0 = wp.tile([P, C], f32)
            nc.sync.dma_start(wv0[:], wv.ap()[0:P, :])
            wv1 = wp.tile([P, C], f32)
            nc.sync.dma_start(wv1[:], wv.ap()[P : 2 * P, :])
            wo0 = wp.tile([P, C], f32)
            nc.sync.dma_start(wo0[:], wout.ap()[0:P, :])
            wo1 = wp.tile([P, C], f32)
            nc.sync.dma_start(wo1[:], wout.ap()[P : 2 * P, :])
            bout_sb = wp.tile([1, C], f32)
            nc.sync.dma_start(bout_sb[:], bout.ap().rearrange("(o c) -> o c", o=1))

            ones_1 = wp.tile([1, P], f32)
            nc.gpsimd.memset(ones_1[:], 1.0)
            ones_big = wp.tile([P, P], f32)
            nc.gpsimd.memset(ones_big[:], 1.0)
            eps_t = wp.tile([1, 1], f32)
            nc.gpsimd.memset(eps_t[:], EPS)

            # ---- img tile loads (stream in parallel with the chain) ----
            imgs = []
            for t_i in range(NT):
                it = iop.tile([P, C], f32, tag="img_in", name=f"img_in_{t_i}")
                nc.sync.dma_start(it[:], img.ap()[t_i * P : (t_i + 1) * P, :])
                imgs.append(it)

            # ---- delta chain (partition layout: x[j*128+p] at [p, j]) ----
            # ctxT = (param @ Wparam)^T  via  Wparam_chunk^T @ param
            ctx_ps = pp.tile([P, 2], f32, tag="ctx_ps")
            nc.tensor.matmul(
                ctx_ps[:, 0:1], wparam_sb[:, 0:P], pT[:], start=True, stop=True
            )
            nc.tensor.matmul(
                ctx_ps[:, 1:2], wparam_sb[:, P:C], pT[:], start=True, stop=True
            )

            # stats_in = [ctxT + bparamT | (ctxT + bparamT)^2]
            stats_in = wp.tile([P, 4], f32)
            nc.vector.tensor_add(stats_in[:, 0:2], ctx_ps[:], bparamT[:])
            nc.vector.tensor_mul(stats_in[:, 2:4], stats_in[:, 0:2], stats_in[:, 0:2])

            # cross-partition sums via ones-matmul: [1,4] = (s_x0, s_x1, s_q0, s_q1)
            sums_ps = pp.tile([1, 4], f32, tag="sums_ps")
            nc.tensor.matmul(
                sums_ps[:], ones_big[:, 0:1], stats_in[:], start=True, stop=True
            )
            sums_sb = wp.tile([1, 4], f32)
            nc.vector.tensor_copy(sums_sb[:], sums_ps[:])

            msum = wp.tile([1, 2], f32)
            nc.vector.tensor_add(msum[:, 0:1], sums_sb[:, 0:1], sums_sb[:, 1:2])
            nc.vector.tensor_add(msum[:, 1:2], sums_sb[:, 2:3], sums_sb[:, 3:4])
            me = wp.tile([1, 2], f32)  # (mean, E[x^2])
            nc.scalar.mul(me[:], msum[:], 1.0 / C)
            m2 = wp.tile([1, 1], f32)
            nc.vector.tensor_mul(m2[:], me[:, 0:1], me[:, 0:1])
            var = wp.tile([1, 1], f32)
            nc.vector.tensor_sub(var[:], me[:, 1:2], m2[:])
            sd = wp.tile([1, 1], f32)
            nc.scalar.activation(sd[:], var[:], AF.Sqrt, bias=eps_t[:])
            rstd = wp.tile([1, 1], f32)
            nc.vector.reciprocal(rstd[:], sd[:])
            mrs = wp.tile([1, 1], f32)
            nc.vector.tensor_mul(mrs[:], me[:, 0:1], rstd[:])
            ab = wp.tile([1, 2], f32)  # (rstd, mean*rstd)
            nc.vector.tensor_copy(ab[:, 0:1], rstd[:])
            nc.vector.tensor_copy(ab[:, 1:2], mrs[:])

            # broadcast (rstd, mean*rstd) to all partitions via K=1 matmul
            bc_ps = pp.tile([P, 2], f32, tag="bc_ps")
            nc.tensor.matmul(bc_ps[:], ones_1[:], ab[:], start=True, stop=True)
            bc = wp.tile([P, 2], f32)
            nc.vector.tensor_copy(bc[:], bc_ps[:])

            # cnT = (ctxT * rstd - mean*rstd) * gT + bT
            xn = wp.tile([P, 2], f32)
            nc.vector.tensor_scalar(
                xn[:],
                stats_in[:, 0:2],
                bc[:, 0:1],
                bc[:, 1:2],
                op0=ALU.mult,
                op1=ALU.subtract,
            )
            tmpg = wp.tile([P, 2], f32)
            nc.vector.tensor_mul(tmpg[:], xn[:], gT[:])
            cnT = wp.tile([P, 2], f32)
            nc.vector.tensor_add(cnT[:], tmpg[:], bT[:])

            # vT[p, j] = v[j*128+p] = sum_k cn[k] * Wv[k, j*128+p]
            vt_ps = pp.tile([P, 2], f32, tag="vt_ps")
            nc.tensor.matmul(vt_ps[:, 0:1], wv0[:, 0:P], cnT[:, 0:1], start=True, stop=False)
            nc.tensor.matmul(vt_ps[:, 0:1], wv1[:, 0:P], cnT[:, 1:2], start=False, stop=True)
            nc.tensor.matmul(vt_ps[:, 1:2], wv0[:, P:C], cnT[:, 0:1], start=True, stop=False)
            nc.tensor.matmul(vt_ps[:, 1:2], wv1[:, P:C], cnT[:, 1:2], start=False, stop=True)
            vT = wp.tile([P, 2], f32)
            nc.vector.tensor_copy(vT[:], vt_ps[:])

            # vrep_j[k, m] = v[j*128+k] replicated along free dim
            vrep0 = wp.tile([P, P], f32)
            nc.vector.tensor_scalar_mul(vrep0[:], ones_big[:], vT[:, 0:1])
            vrep1 = wp.tile([P, P], f32)
            nc.scalar.mul(vrep1[:], ones_big[:], vT[:, 1:2])

            # delta (already broadcast across partitions):
            # delta[p, c] = sum_k v[k] Wout[k, c] + bout[c]
            delta_ps = pp.tile([P, C], f32, tag="delta_ps")
            nc.tensor.matmul(delta_ps[:], vrep0[:], wo0[:], start=True, stop=False)
            nc.tensor.matmul(delta_ps[:], vrep1[:], wo1[:], start=False, stop=False)
            nc.tensor.matmul(delta_ps[:], ones_1[:], bout_sb[:], start=False, stop=True)
            delta = wp.tile([P, C], f32)
            nc.vector.tensor_copy(delta[:], delta_ps[:])

            # ---- stream: out_tile = img_tile + delta ----
            for t_i in range(NT):
                ot = iop.tile([P, C], f32, tag="img_out", name=f"img_out_{t_i}")
                nc.vector.tensor_add(ot[:], imgs[t_i][:], delta[:])
                nc.sync.dma_start(out.ap()[t_i * P : (t_i + 1) * P, :], ot[:])

    nc.compile()
    return nc


def get_nc():
    global _BUILT
    if _BUILT is None:
        _BUILT = _build()
    return _BUILT


def kernel(**inputs):
    from concourse.bass_utils import run_bass_kernel_spmd

    nc = get_nc()

    f = lambda a: np.ascontiguousarray(np.asarray(a, dtype=np.float32))
    img = f(inputs["img_tokens"])          # [B, N, C]
    param = f(inputs["param_tokens"])      # [B, 16]
    wparam = f(inputs["Wparam"])           # [16, C]
    bparam = f(inputs["bparam"])           # [C]
    gln = f(inputs["ctx_norm_g"])          # [C]
    bln = f(inputs["ctx_norm_b"])          # [C]
    wv = f(np.asarray(inputs["Wkv"], dtype=np.float32)[:, C:])  # v-half [C, C]
    wout = f(inputs["Wout"])               # [C, C]
    bout = f(inputs["bout"])               # [C]

    in_maps = [
        {
            "img": img[b],
            "param": param[b],
            "wparam": wparam,
            "bparam": bparam,
            "gln": gln,
            "bln": bln,
            "wv": wv,
            "wout": wout,
            "bout": bout,
        }
        for b in range(NCORES)
    ]
    res = run_bass_kernel_spmd(nc, in_maps, core_ids=list(range(NCORES)))
    out = np.stack([res.results[b]["out"] for b in range(NCORES)], axis=0)
    return out.astype(np.float32)


# revision 5
# speedup vs baseline: 1.1536x; 1.1536x over previous
"""Trainium2 Bass kernel for nn_CrossAttentionBlock_78881369358733.

The reference block's attention is degenerate: every query attends to a
single broadcast context token, so softmax over N identical scores is
exactly uniform and the attention output equals `v` for every position.
The whole module collapses to

    ctx   = param_tokens @ Wparam + bparam          # [B, C]
    v     = layernorm(ctx) @ Wkv[:, C:]             # [B, C]
    delta = v @ Wout + bout                         # [B, C]
    out   = img_tokens + delta[:, None, :]          # [B, N, C]

(q, Wq, img layernorm params, and the k-half of Wkv are dead.)

Sharding: pure data parallel over B — core b handles batch b. Each core
computes its own tiny delta vector on-device (PE matmuls + DVE/ACT ops)
and streams img tiles through a broadcast add.

DMA design (cost-model-driven): the HWDGE ring charges ~625ns per
dma_start regardless of size and the DMA engines are a shared ~360GB/s
pipe, so transfers are batched into few large DMAs: one packed
[Wparam|param] array, one packed [bparamT|gT|bT] array, single-DMA
wv/wout loads, 2x 512KB img loads and 4x 256KB stores.
"""

import sys

if "/opt/trn_rl_repo" not in sys.path:
    sys.path.append("/opt/trn_rl_repo")

import numpy as np

B, N, C = 8, 1024, 256
PARAM_DIM = 16
EPS = 1e-5
P = 128
NBLK = N // P  # 128-row blocks per core (8)
NCORES = 8

_BUILT = None


def _patch_tile_tail(tile_mod):
    """The stock TileContext tail emits a drain with one sem-wait per live
    proc (rejected by this walrus: too many sync waits per TPB_CTRL)
    followed by an EVSEM barrier + sem reset that faults the exec unit on
    this runtime. A single drain is sufficient for one-shot NEFF execution:
    semaphores are re-initialized by each nrt_execute."""
    from bass_rust import ScopedClock

    def _drain_only(self, tick_clock, wait_clock):
        drain_inst = self.nc.sync.drain()
        wait_clock.add_sem_waits(
            drain_inst.ins, ScopedClock({None: tick_clock.global_clock})
        )
        popped = self.nc._tile_sem_poison_stack.pop()
        assert popped is self._sem_poison

    tile_mod.TileContext._drain_and_barrier = _drain_only


def _build():
    import concourse.bacc as bacc
    import concourse.tile as tile
    from concourse import mybir

    _patch_tile_tail(tile)

    f32 = mybir.dt.float32
    AF = mybir.ActivationFunctionType
    ALU = mybir.AluOpType

    nc = bacc.Bacc("TRN2", target_bir_lowering=False, debug=False)

    img = nc.dram_tensor("img", [N, C], f32, kind="ExternalInput")
    # wp_pk[:, 0:C] = Wparam, wp_pk[:, C] = param_tokens[b]
    wp_pk = nc.dram_tensor("wp_pk", [PARAM_DIM, C + 1], f32, kind="ExternalInput")
    # vecs[p, 0:2|2:4|4:6] = bparam|ctx_g|ctx_b in [128, 2] partition layout
    vecs = nc.dram_tensor("vecs", [P, 6], f32, kind="ExternalInput")
    wv = nc.dram_tensor("wv", [C, C], f32, kind="ExternalInput")
    wout = nc.dram_tensor("wout", [C, C], f32, kind="ExternalInput")
    bout = nc.dram_tensor("bout", [C], f32, kind="ExternalInput")
    out = nc.dram_tensor("out", [N, C], f32, kind="ExternalOutput")

    with tile.TileContext(nc) as tc:
        with (
            tc.tile_pool(name="w", bufs=1) as wp,
            tc.tile_pool(name="io", bufs=1) as iop,
            tc.tile_pool(name="ps", bufs=1, space="PSUM") as pp,
        ):
            # ---- loads: ACT ring gets the small chain-critical arrays ----
            wp_sb = wp.tile([PARAM_DIM, C + 1], f32)
            nc.scalar.dma_start(wp_sb[:], wp_pk.ap())
            vecs_sb = wp.tile([P, 6], f32)
            nc.scalar.dma_start(vecs_sb[:], vecs.ap())
            bout_sb = wp.tile([1, C], f32)
            nc.scalar.dma_start(bout_sb[:], bout.ap().rearrange("(o c) -> o c", o=1))

            # ---- SP ring: the big loads, in chain-dependency order ----
            # wv_sb[p, j*C + c] = Wv[j*128 + p, c] (one DMA per matrix)
            wv_sb = wp.tile([P, 2 * C], f32)
            nc.sync.dma_start(
                wv_sb[:].rearrange("p (j c) -> p j c", j=2),
                wv.ap().rearrange("(j p) c -> p j c", p=P),
            )
            wo_sb = wp.tile([P, 2 * C], f32)
            nc.sync.dma_start(
                wo_sb[:].rearrange("p (j c) -> p j c", j=2),
                wout.ap().rearrange("(j p) c -> p j c", p=P),
            )
            # img: 2 DMAs of 512KB; imgL[L][p, j*C+c] = img[L*512 + j*128 + p, c]
            imgL = []
            for L in range(2):
                t = iop.tile([P, 4 * C], f32, tag=f"img_in{L}", name=f"img_in_{L}")
                nc.sync.dma_start(
                    t[:].rearrange("p (j c) -> p j c", j=4),
                    img.ap()[L * 512 : (L + 1) * 512, :].rearrange(
                        "(j p) c -> p j c", p=P
                    ),
                )
                imgL.append(t)

            # ---- constants (DVE memsets; Pool stays idle) ----
            ones_1 = wp.tile([1, P], f32)
            nc.vector.memset(ones_1[:], 1.0)
            ones_big = wp.tile([P, P], f32)
            nc.vector.memset(ones_big[:], 1.0)
            invC_col = wp.tile([P, 1], f32)
            nc.vector.memset(invC_col[:], 1.0 / C)
            eps_t = wp.tile([1, 1], f32)
            nc.vector.memset(eps_t[:], EPS)

            # ---- delta chain (partition layout: x[j*128+p] at [p, j]) ----
            # ctxT = (param @ Wparam)^T  via  Wparam_chunk^T @ param
            ctx_ps = pp.tile([P, 2], f32, tag="ctx_ps")
            nc.tensor.matmul(
                ctx_ps[:, 0:1], wp_sb[:, 0:P], wp_sb[:, C : C + 1], start=True, stop=True
            )
            nc.tensor.matmul(
                ctx_ps[:, 1:2], wp_sb[:, P:C], wp_sb[:, C : C + 1], start=True, stop=True
            )

            # stats_in = [ctxT + bparamT | (ctxT + bparamT)^2]
            stats_in = wp.tile([P, 4], f32)
            nc.vector.tensor_add(stats_in[:, 0:2], ctx_ps[:], vecs_sb[:, 0:2])
            nc.vector.tensor_mul(stats_in[:, 2:4], stats_in[:, 0:2], stats_in[:, 0:2])

            # partition sums scaled by 1/C: (mx0, mx1, mq0, mq1)
            sums_ps = pp.tile([1, 4], f32, tag="sums_ps")
            nc.tensor.matmul(sums_ps[:], invC_col[:], stats_in[:], start=True, stop=True)
            sums_sb = wp.tile([1, 4], f32)
            nc.vector.tensor_copy(sums_sb[:], sums_ps[:])

            moms = wp.tile([1, 2], f32)  # (mean, E[x^2])
            nc.vector.tensor_add(moms[:, 0:1], sums_sb[:, 0:1], sums_sb[:, 1:2])
            nc.vector.tensor_add(moms[:, 1:2], sums_sb[:, 2:3], sums_sb[:, 3:4])
            m2 = wp.tile([1, 1], f32)
            nc.vector.tensor_mul(m2[:], moms[:, 0:1], moms[:, 0:1])
            var = wp.tile([1, 1], f32)
            nc.vector.tensor_sub(var[:], moms[:, 1:2], m2[:])
            sd = wp.tile([1, 1], f32)
            nc.scalar.activation(sd[:], var[:], AF.Sqrt, bias=eps_t[:])
            ab = wp.tile([1, 2], f32)  # (rstd, mean*rstd)
            nc.vector.reciprocal(ab[:, 0:1], sd[:])
            nc.vector.tensor_mul(ab[:, 1:2], moms[:, 0:1], ab[:, 0:1])

            # broadcast (rstd, mean*rstd) to all partitions via K=1 matmul
            bc_ps = pp.tile([P, 2], f32, tag="bc_ps")
            nc.tensor.matmul(bc_ps[:], ones_1[:], ab[:], start=True, stop=True)
            bc = wp.tile([P, 2], f32)
            nc.vector.tensor_copy(bc[:], bc_ps[:])

            # cnT = (ctxT * rstd - mean*rstd) * gT + bT
            xn = wp.tile([P, 2], f32)
            nc.vector.tensor_scalar(
                xn[:],
                stats_in[:, 0:2],
                bc[:, 0:1],
                bc[:, 1:2],
                op0=ALU.mult,
                op1=ALU.subtract,
            )
            tmpg = wp.tile([P, 2], f32)
            nc.vector.tensor_mul(tmpg[:], xn[:], vecs_sb[:, 2:4])
            cnT = wp.tile([P, 2], f32)
            nc.vector.tensor_add(cnT[:], tmpg[:], vecs_sb[:, 4:6])

            # vT[p, j] = v[j*128+p] = sum_k cn[k] * Wv[k, j*128+p]
            vt_ps = pp.tile([P, 2], f32, tag="vt_ps")
            nc.tensor.matmul(vt_ps[:, 0:1], wv_sb[:, 0:P], cnT[:, 0:1], start=True, stop=False)
            nc.tensor.matmul(vt_ps[:, 0:1], wv_sb[:, C : C + P], cnT[:, 1:2], start=False, stop=True)
            nc.tensor.matmul(vt_ps[:, 1:2], wv_sb[:, P:C], cnT[:, 0:1], start=True, stop=False)
            nc.tensor.matmul(vt_ps[:, 1:2], wv_sb[:, C + P : 2 * C], cnT[:, 1:2], start=False, stop=True)
            vT = wp.tile([P, 2], f32)
            nc.vector.tensor_copy(vT[:], vt_ps[:])

            # vrep_j[k, m] = v[j*128+k] replicated along free dim
            vrep0 = wp.tile([P, P], f32)
            nc.vector.tensor_scalar_mul(vrep0[:], ones_big[:], vT[:, 0:1])
            vrep1 = wp.tile([P, P], f32)
            nc.vector.tensor_scalar_mul(vrep1[:], ones_big[:], vT[:, 1:2])

            # delta[p, c] = sum_k v[k] Wout[k, c] + bout[c]  (all partitions)
            delta_ps = pp.tile([P, C], f32, tag="delta_ps")
            nc.tensor.matmul(delta_ps[:], vrep0[:], wo_sb[:, 0:C], start=True, stop=False)
            nc.tensor.matmul(delta_ps[:], vrep1[:], wo_sb[:, C : 2 * C], start=False, stop=False)
            nc.tensor.matmul(delta_ps[:], ones_1[:], bout_sb[:], start=False, stop=True)

            # ---- stream: out = img + delta, 4 stores of [128, 512] ----
            for k in range(4):
                ot = iop.tile([P, 2 * C], f32, tag="img_out", bufs=4, name=f"ot_{k}")
                for h in range(2):
                    blk = 2 * k + h
                    L, off = blk // 4, (blk % 4) * C
                    nc.vector.tensor_add(
                        ot[:, h * C : (h + 1) * C],
                        imgL[L][:, off : off + C],
                        delta_ps[:],
                    )
                eng = nc.sync if k % 2 == 0 else nc.scalar
                eng.dma_start(
                    out.ap()[k * 256 : (k + 1) * 256, :].rearrange(
                        "(j p) c -> p j c", p=P
                    ),
                    ot[:].rearrange("p (j c) -> p j c", j=2),
                )

    nc.compile()
    return nc


def get_nc():
    global _BUILT
    if _BUILT is None:
        _BUILT = _build()
    return _BUILT


def _pack_inputs(inputs):
    f = lambda a: np.ascontiguousarray(np.asarray(a, dtype=np.float32))
    img = f(inputs["img_tokens"])  # [B, N, C]
    param = f(inputs["param_tokens"])  # [B, 16]
    wparam = f(inputs["Wparam"])  # [16, C]
    bparam = f(inputs["bparam"])  # [C]
    gln = f(inputs["ctx_norm_g"])  # [C]
    bln = f(inputs["ctx_norm_b"])  # [C]
    wv = f(np.asarray(inputs["Wkv"], dtype=np.float32)[:, C:])  # [C, C]
    wout = f(inputs["Wout"])  # [C, C]
    bout = f(inputs["bout"])  # [C]

    # vecs[p, 2*t + j] = tensor_t[j*128 + p]
    vecs = np.empty((P, 6), np.float32)
    for t_i, vec in enumerate((bparam, gln, bln)):
        m = vec.reshape(2, P).T  # [128, 2]
        vecs[:, 2 * t_i : 2 * t_i + 2] = m
    vecs = np.ascontiguousarray(vecs)

    in_maps = []
    for b in range(NCORES):
        wp_pk = np.empty((PARAM_DIM, C + 1), np.float32)
        wp_pk[:, :C] = wparam
        wp_pk[:, C] = param[b]
        in_maps.append(
            {
                "img": img[b],
                "wp_pk": np.ascontiguousarray(wp_pk),
                "vecs": vecs,
                "wv": wv,
                "wout": wout,
                "bout": bout,
            }
        )
    return in_maps


def kernel(**inputs):
    from concourse.bass_utils import run_bass_kernel_spmd

    nc = get_nc()
    in_maps = _pack_inputs(inputs)
    res = run_bass_kernel_spmd(nc, in_maps, core_ids=list(range(NCORES)))
    out = np.stack([res.results[b]["out"] for b in range(NCORES)], axis=0)
    return out.astype(np.float32)


# revision 9
# speedup vs baseline: 1.2484x; 1.0822x over previous
"""Trainium2 Bass kernel for nn_CrossAttentionBlock_78881369358733.

The reference block's attention is degenerate: every query attends to a
single broadcast context token, so softmax over N identical scores is
exactly uniform and the attention output equals `v` for every position.
The whole module collapses to

    ctx   = param_tokens @ Wparam + bparam          # [B, C]
    v     = layernorm(ctx) @ Wkv[:, C:]             # [B, C]
    delta = v @ Wout + bout                         # [B, C]
    out   = img_tokens + delta[:, None, :]          # [B, N, C]

(q, Wq, img layernorm params, and the k-half of Wkv are dead.)

Sharding: pure data parallel over B — core b handles batch b. Each core
computes its own tiny delta vector on-device (PE matmuls + DVE/ACT ops)
and streams img tiles through a broadcast add.

DMA design (cost-model-driven): the HWDGE ring charges ~625ns per
dma_start regardless of size and the DMA engines are a shared ~360GB/s
pipe, so transfers are batched into few large DMAs: one packed
[Wparam|param] array, one packed [bparamT|gT|bT] array, single-DMA
wv/wout loads, 2x 512KB img loads and 4x 256KB stores.
"""

import sys

if "/opt/trn_rl_repo" not in sys.path:
    sys.path.append("/opt/trn_rl_repo")

import numpy as np

B, N, C = 8, 1024, 256
PARAM_DIM = 16
EPS = 1e-5
P = 128
NBLK = N // P  # 128-row blocks per core (8)
NCORES = 8

_BUILT = None


def _patch_tile_tail(tile_mod):
    """The stock TileContext tail emits a drain with one sem-wait per live
    proc (rejected by this walrus: too many sync waits per TPB_CTRL)
    followed by an EVSEM barrier + sem reset that faults the exec unit on
    this runtime. A single drain is sufficient for one-shot NEFF execution:
    semaphores are re-initialized by each nrt_execute."""
    from bass_rust import ScopedClock

    def _drain_only(self, tick_clock, wait_clock):
        drain_inst = self.nc.sync.drain()
        wait_clock.add_sem_waits(
            drain_inst.ins, ScopedClock({None: tick_clock.global_clock})
        )
        popped = self.nc._tile_sem_poison_stack.pop()
        assert popped is self._sem_poison

    tile_mod.TileContext._drain_and_barrier = _drain_only


def _build():
    import concourse.bacc as bacc
    import concourse.tile as tile
    from concourse import mybir

    _patch_tile_tail(tile)

    f32 = mybir.dt.float32
    AF = mybir.ActivationFunctionType
    ALU = mybir.AluOpType

    nc = bacc.Bacc("TRN2", target_bir_lowering=False, debug=False)

    img = nc.dram_tensor("img", [N, C], f32, kind="ExternalInput")
    # wp_pk[:, 0:C] = Wparam, wp_pk[:, C] = param_tokens[b]
    wp_pk = nc.dram_tensor("wp_pk", [PARAM_DIM, C + 1], f32, kind="ExternalInput")
    # vecs[p, 0:2|2:4|4:6] = bparam|ctx_g|ctx_b in [128, 2] partition layout
    vecs = nc.dram_tensor("vecs", [P, 6], f32, kind="ExternalInput")
    wv = nc.dram_tensor("wv", [C, C], f32, kind="ExternalInput")
    wout = nc.dram_tensor("wout", [C, C], f32, kind="ExternalInput")
    bout = nc.dram_tensor("bout", [C], f32, kind="ExternalInput")
    out = nc.dram_tensor("out", [N, C], f32, kind="ExternalOutput")

    with tile.TileContext(nc) as tc:
        with (
            tc.tile_pool(name="w", bufs=1) as wp,
            tc.tile_pool(name="io", bufs=1) as iop,
            tc.tile_pool(name="ps", bufs=1, space="PSUM") as pp,
        ):
            # ---- SP ring loads, in chain-dependency order (ACT kept
            # DMA-free so its activation-table load runs immediately) ----
            wp_sb = wp.tile([PARAM_DIM, C + 1], f32)
            nc.sync.dma_start(wp_sb[:], wp_pk.ap())
            vecs_sb = wp.tile([P, 6], f32)
            nc.sync.dma_start(vecs_sb[:], vecs.ap())
            # wv_sb[p, j*C + c] = Wv[j*128 + p, c] (one DMA per matrix)
            wv_sb = wp.tile([P, 2 * C], f32)
            nc.sync.dma_start(
                wv_sb[:].rearrange("p (j c) -> p j c", j=2),
                wv.ap().rearrange("(j p) c -> p j c", p=P),
            )
            wo_sb = wp.tile([P, 2 * C], f32)
            nc.sync.dma_start(
                wo_sb[:].rearrange("p (j c) -> p j c", j=2),
                wout.ap().rearrange("(j p) c -> p j c", p=P),
            )
            bout_sb = wp.tile([1, C], f32)
            nc.sync.dma_start(bout_sb[:], bout.ap().rearrange("(o c) -> o c", o=1))
            # img: 2 DMAs of 512KB; imgL[L][p, j*C+c] = img[L*512 + j*128 + p, c]
            imgL = []
            for L in range(2):
                t = iop.tile([P, 4 * C], f32, tag=f"img_in{L}", name=f"img_in_{L}")
                nc.sync.dma_start(
                    t[:].rearrange("p (j c) -> p j c", j=4),
                    img.ap()[L * 512 : (L + 1) * 512, :].rearrange(
                        "(j p) c -> p j c", p=P
                    ),
                )
                imgL.append(t)

            # ---- constants (DVE memsets; Pool stays idle) ----
            ones_1 = wp.tile([1, P], f32)
            nc.vector.memset(ones_1[:], 1.0)
            ones_big = wp.tile([P, P], f32)
            nc.vector.memset(ones_big[:], 1.0)
            invC_col = wp.tile([P, 1], f32)
            nc.vector.memset(invC_col[:], 1.0 / C)
            eps_t = wp.tile([1, 1], f32)
            nc.vector.memset(eps_t[:], EPS)

            # ---- delta chain (partition layout: x[j*128+p] at [p, j]) ----
            # ctxT = (param @ Wparam)^T  via  Wparam_chunk^T @ param
            ctx_ps = pp.tile([P, 2], f32, tag="ctx_ps")
            nc.tensor.matmul(
                ctx_ps[:, 0:1], wp_sb[:, 0:P], wp_sb[:, C : C + 1], start=True, stop=True
            )
            nc.tensor.matmul(
                ctx_ps[:, 1:2], wp_sb[:, P:C], wp_sb[:, C : C + 1], start=True, stop=True
            )

            # stats_in interleaved: cols (x0, x^2_0, x1, x^2_1)
            stats_in = wp.tile([P, 4], f32)
            nc.vector.tensor_add(stats_in[:, 0:4:2], ctx_ps[:], vecs_sb[:, 0:2])
            nc.vector.tensor_mul(
                stats_in[:, 1:4:2], stats_in[:, 0:4:2], stats_in[:, 0:4:2]
            )

            # partition sums scaled by 1/C: (mx0, mq0, mx1, mq1)
            sums_ps = pp.tile([1, 4], f32, tag="sums_ps")
            nc.tensor.matmul(sums_ps[:], invC_col[:], stats_in[:], start=True, stop=True)
            sums_sb = wp.tile([1, 4], f32)
            nc.vector.tensor_copy(sums_sb[:], sums_ps[:])

            moms = wp.tile([1, 2], f32)  # (mean, E[x^2])
            nc.vector.tensor_add(moms[:], sums_sb[:, 0:2], sums_sb[:, 2:4])
            m2 = wp.tile([1, 1], f32)
            nc.vector.tensor_mul(m2[:], moms[:, 0:1], moms[:, 0:1])
            var = wp.tile([1, 1], f32)
            nc.vector.tensor_sub(var[:], moms[:, 1:2], m2[:])
            sd = wp.tile([1, 1], f32)
            nc.scalar.activation(sd[:], var[:], AF.Sqrt, bias=eps_t[:])
            ab = wp.tile([1, 2], f32)  # (rstd, mean*rstd)
            nc.vector.reciprocal(ab[:, 0:1], sd[:])
            nc.vector.tensor_mul(ab[:, 1:2], moms[:, 0:1], ab[:, 0:1])

            # broadcast (rstd, mean*rstd) to all partitions via K=1 matmul
            bc_ps = pp.tile([P, 2], f32, tag="bc_ps")
            nc.tensor.matmul(bc_ps[:], ones_1[:], ab[:], start=True, stop=True)
            bc = wp.tile([P, 2], f32)
            nc.vector.tensor_copy(bc[:], bc_ps[:])

            # cnT = (ctxT * rstd - mean*rstd) * gT + bT
            xn = wp.tile([P, 2], f32)
            nc.vector.tensor_scalar(
                xn[:],
                stats_in[:, 0:4:2],
                bc[:, 0:1],
                bc[:, 1:2],
                op0=ALU.mult,
                op1=ALU.subtract,
            )
            tmpg = wp.tile([P, 2], f32)
            nc.vector.tensor_mul(tmpg[:], xn[:], vecs_sb[:, 2:4])
            cnT = wp.tile([P, 2], f32)
            nc.vector.tensor_add(cnT[:], tmpg[:], vecs_sb[:, 4:6])

            # vT[p, j] = v[j*128+p] = sum_k cn[k] * Wv[k, j*128+p]
            vt_ps = pp.tile([P, 2], f32, tag="vt_ps")
            nc.tensor.matmul(vt_ps[:, 0:1], wv_sb[:, 0:P], cnT[:, 0:1], start=True, stop=False)
            nc.tensor.matmul(vt_ps[:, 0:1], wv_sb[:, C : C + P], cnT[:, 1:2], start=False, stop=True)
            nc.tensor.matmul(vt_ps[:, 1:2], wv_sb[:, P:C], cnT[:, 0:1], start=True, stop=False)
            nc.tensor.matmul(vt_ps[:, 1:2], wv_sb[:, C + P : 2 * C], cnT[:, 1:2], start=False, stop=True)

            # vrep_j[k, m] = v[j*128+k] replicated along free dim (scalar
            # operand read straight from PSUM)
            vrep0 = wp.tile([P, P], f32)
            nc.vector.tensor_scalar_mul(vrep0[:], ones_big[:], vt_ps[:, 0:1])
            vrep1 = wp.tile([P, P], f32)
            nc.vector.tensor_scalar_mul(vrep1[:], ones_big[:], vt_ps[:, 1:2])

            # delta[p, c] = sum_k v[k] Wout[k, c] + bout[c]  (all partitions)
            # bias term first: it only needs bout, so PE runs it before vT
            # is ready.
            delta_ps = pp.tile([P, C], f32, tag="delta_ps")
            nc.tensor.matmul(delta_ps[:], ones_1[:], bout_sb[:], start=True, stop=False)
            nc.tensor.matmul(delta_ps[:], vrep0[:], wo_sb[:, 0:C], start=False, stop=False)
            nc.tensor.matmul(delta_ps[:], vrep1[:], wo_sb[:, C : 2 * C], start=False, stop=True)

            # delta duplicated side by side for [128, 512] adds
            delta2 = wp.tile([P, 2 * C], f32)
            nc.vector.tensor_copy(delta2[:, 0:C], delta_ps[:])
            nc.vector.tensor_copy(delta2[:, C : 2 * C], delta_ps[:])

            # ---- stream: out = img + delta, 4 adds + 4 stores of [128, 512]
            for k in range(4):
                ot = iop.tile([P, 2 * C], f32, tag="img_out", bufs=4, name=f"ot_{k}")
                L, off = k // 2, (k % 2) * 2 * C
                nc.vector.tensor_add(
                    ot[:], imgL[L][:, off : off + 2 * C], delta2[:]
                )
                nc.scalar.dma_start(
                    out.ap()[k * 256 : (k + 1) * 256, :].rearrange(
                        "(j p) c -> p j c", p=P
                    ),
                    ot[:].rearrange("p (j c) -> p j c", j=2),
                )

    nc.compile()
    return nc


def get_nc():
    global _BUILT
    if _BUILT is None:
        _BUILT = _build()
    return _BUILT


def _pack_inputs(inputs):
    f = lambda a: np.ascontiguousarray(np.asarray(a, dtype=np.float32))
    img = f(inputs["img_tokens"])  # [B, N, C]
    param = f(inputs["param_tokens"])  # [B, 16]
    wparam = f(inputs["Wparam"])  # [16, C]
    bparam = f(inputs["bparam"])  # [C]
    gln = f(inputs["ctx_norm_g"])  # [C]
    bln = f(inputs["ctx_norm_b"])  # [C]
    wv = f(np.asarray(inputs["Wkv"], dtype=np.float32)[:, C:])  # [C, C]
    wout = f(inputs["Wout"])  # [C, C]
    bout = f(inputs["bout"])  # [C]

    # vecs[p, 2*t + j] = tensor_t[j*128 + p]
    vecs = np.empty((P, 6), np.float32)
    for t_i, vec in enumerate((bparam, gln, bln)):
        m = vec.reshape(2, P).T  # [128, 2]
        vecs[:, 2 * t_i : 2 * t_i + 2] = m
    vecs = np.ascontiguousarray(vecs)

    in_maps = []
    for b in range(NCORES):
        wp_pk = np.empty((PARAM_DIM, C + 1), np.float32)
        wp_pk[:, :C] = wparam
        wp_pk[:, C] = param[b]
        in_maps.append(
            {
                "img": img[b],
                "wp_pk": np.ascontiguousarray(wp_pk),
                "vecs": vecs,
                "wv": wv,
                "wout": wout,
                "bout": bout,
            }
        )
    return in_maps


def kernel(**inputs):
    from concourse.bass_utils import run_bass_kernel_spmd

    nc = get_nc()
    in_maps = _pack_inputs(inputs)
    res = run_bass_kernel_spmd(nc, in_maps, core_ids=list(range(NCORES)))
    out = np.stack([res.results[b]["out"] for b in range(NCORES)], axis=0)
    return out.astype(np.float32)


# revision 11
# speedup vs baseline: 1.3043x; 1.0448x over previous
"""Trainium2 Bass kernel for nn_CrossAttentionBlock_78881369358733.

The reference block's attention is degenerate: every query attends to a
single broadcast context token, so softmax over N identical scores is
exactly uniform and the attention output equals `v` for every position.
The whole module collapses to

    ctx   = param_tokens @ Wparam + bparam          # [B, C]
    v     = layernorm(ctx) @ Wkv[:, C:]             # [B, C]
    delta = v @ Wout + bout                         # [B, C]
    out   = img_tokens + delta[:, None, :]          # [B, N, C]

(q, Wq, img layernorm params, and the k-half of Wkv are dead.)

Sharding: pure data parallel over B — core b handles batch b. Each core
computes its own tiny delta vector on-device (PE matmuls + DVE/ACT ops)
and streams img tiles through a broadcast add.

Perf notes (cost-model-driven):
- each dma_start costs ~625ns on the shared HWDGE ring + ~900ns sem
  propagation, so small tensors are host-packed into one SWDGE-loaded
  array and img moves in few large DMAs;
- LN moments are broadcast to all 128 partitions with a ones*(1/C)
  matmul so every later step uses cheap per-partition scalars;
- the delta matmuls run as float32r (full-rate fp32 PE mode).
"""

import sys

if "/opt/trn_rl_repo" not in sys.path:
    sys.path.append("/opt/trn_rl_repo")

import numpy as np

B, N, C = 8, 1024, 256
PARAM_DIM = 16
EPS = 1e-5
P = 128
NCORES = 8
USE_F32R = False

# smalls layout: [128, 264]
#   [0:16, 0:256]  Wparam
#   [0:16, 256]    param_tokens[b]
#   [:, 257:259]   bparam as [128, 2]  (x[j*128+p] at [p, j])
#   [:, 259:261]   ctx_norm_g likewise
#   [:, 261:263]   ctx_norm_b likewise
SMALLS_F = 264

_BUILT = None


def _patch_tile_tail(tile_mod):
    """The stock TileContext tail emits a drain with one sem-wait per live
    proc (rejected by this walrus: too many sync waits per TPB_CTRL)
    followed by an EVSEM barrier + sem reset that faults the exec unit on
    this runtime. A single drain is sufficient for one-shot NEFF execution:
    semaphores are re-initialized by each nrt_execute."""
    from bass_rust import ScopedClock

    def _drain_only(self, tick_clock, wait_clock):
        drain_inst = self.nc.sync.drain()
        wait_clock.add_sem_waits(
            drain_inst.ins, ScopedClock({None: tick_clock.global_clock})
        )
        popped = self.nc._tile_sem_poison_stack.pop()
        assert popped is self._sem_poison

    tile_mod.TileContext._drain_and_barrier = _drain_only


def _build():
    import concourse.bacc as bacc
    import concourse.tile as tile
    from concourse import mybir

    _patch_tile_tail(tile)

    f32 = mybir.dt.float32
    f32r = mybir.dt.float32r
    AF = mybir.ActivationFunctionType
    ALU = mybir.AluOpType

    def mmcast(ap):
        return ap.bitcast(f32r) if USE_F32R else ap

    nc = bacc.Bacc("TRN2", target_bir_lowering=False, debug=False)

    img = nc.dram_tensor("img", [N, C], f32, kind="ExternalInput")
    smalls = nc.dram_tensor("smalls", [P, SMALLS_F], f32, kind="ExternalInput")
    wv = nc.dram_tensor("wv", [C, C], f32, kind="ExternalInput")
    wout = nc.dram_tensor("wout", [C, C], f32, kind="ExternalInput")
    bout = nc.dram_tensor("bout", [C], f32, kind="ExternalInput")
    out = nc.dram_tensor("out", [N, C], f32, kind="ExternalOutput")

    with tile.TileContext(nc) as tc:
        with (
            tc.tile_pool(name="w", bufs=1) as wp,
            tc.tile_pool(name="io", bufs=1) as iop,
            tc.tile_pool(name="ps", bufs=1, space="PSUM") as pp,
        ):
            # ---- smalls via SWDGE (Pool) — lowest-latency path, and it
            # keeps the HWDGE ring free for the big transfers ----
            sm = wp.tile([P, SMALLS_F], f32)
            nc.gpsimd.dma_start(sm[:], smalls.ap())

            # ---- SP ring loads in chain-dependency order ----
            # wv_sb[p, j*C + c] = Wv[j*128 + p, c] (one DMA per matrix)
            wv_sb = wp.tile([P, 2 * C], f32)
            nc.sync.dma_start(
                wv_sb[:].rearrange("p (j c) -> p j c", j=2),
                wv.ap().rearrange("(j p) c -> p j c", p=P),
            )
            wo_sb = wp.tile([P, 2 * C], f32)
            nc.sync.dma_start(
                wo_sb[:].rearrange("p (j c) -> p j c", j=2),
                wout.ap().rearrange("(j p) c -> p j c", p=P),
            )
            bout_sb = wp.tile([1, C], f32)
            nc.sync.dma_start(bout_sb[:], bout.ap().rearrange("(o c) -> o c", o=1))
            # img: 2 DMAs of 512KB; imgL[L][p, j*C+c] = img[L*512 + j*128 + p, c]
            imgL = []
            for L in range(2):
                t = iop.tile([P, 4 * C], f32, tag=f"img_in{L}", name=f"img_in_{L}")
                nc.sync.dma_start(
                    t[:].rearrange("p (j c) -> p j c", j=4),
                    img.ap()[L * 512 : (L + 1) * 512, :].rearrange(
                        "(j p) c -> p j c", p=P
                    ),
                )
                imgL.append(t)

            # ---- constants (DVE memsets; Pool busy with the smalls DMA) ----
            ones_1 = wp.tile([1, P], f32)
            nc.vector.memset(ones_1[:], 1.0)
            ones_big = wp.tile([P, P], f32)
            nc.vector.memset(ones_big[:], 1.0)
            invC_mat = wp.tile([P, P], f32)
            nc.vector.memset(invC_mat[:], 1.0 / C)
            eps_col = wp.tile([P, 1], f32)
            nc.vector.memset(eps_col[:], EPS)

            # ---- delta chain (partition layout: x[j*128+p] at [p, j]) ----
            # ctxT = (param @ Wparam)^T  via  Wparam_chunk^T @ param
            ctx_ps = pp.tile([P, 2], f32, tag="ctx_ps")
            nc.tensor.matmul(
                ctx_ps[:, 0:1], sm[0:PARAM_DIM, 0:P], sm[0:PARAM_DIM, C : C + 1],
                start=True, stop=True,
            )
            nc.tensor.matmul(
                ctx_ps[:, 1:2], sm[0:PARAM_DIM, P:C], sm[0:PARAM_DIM, C : C + 1],
                start=True, stop=True,
            )

            # stats_in interleaved: cols (x0, x^2_0, x1, x^2_1)
            stats_in = wp.tile([P, 4], f32)
            nc.vector.tensor_add(stats_in[:, 0:4:2], ctx_ps[:], sm[:, 257:259])
            nc.vector.tensor_mul(
                stats_in[:, 1:4:2], stats_in[:, 0:4:2], stats_in[:, 0:4:2]
            )

            # moments broadcast to ALL partitions: ones(1/C)^T @ stats
            sums_ps = pp.tile([P, 4], f32, tag="sums_ps")
            nc.tensor.matmul(sums_ps[:], invC_mat[:], stats_in[:], start=True, stop=True)
            sums_sb = wp.tile([P, 4], f32)
            nc.vector.tensor_copy(sums_sb[:], sums_ps[:])

            moms = wp.tile([P, 2], f32)  # (mean, E[x^2]) on every partition
            nc.vector.tensor_add(moms[:], sums_sb[:, 0:2], sums_sb[:, 2:4])
            m2 = wp.tile([P, 1], f32)
            nc.vector.tensor_mul(m2[:], moms[:, 0:1], moms[:, 0:1])
            var = wp.tile([P, 1], f32)
            nc.vector.tensor_sub(var[:], moms[:, 1:2], m2[:])
            sd = wp.tile([P, 1], f32)
            nc.scalar.activation(sd[:], var[:], AF.Sqrt, bias=eps_col[:])
            rstd = wp.tile([P, 1], f32)
            nc.vector.reciprocal(rstd[:], sd[:])
            mrs = wp.tile([P, 1], f32)
            nc.vector.tensor_mul(mrs[:], moms[:, 0:1], rstd[:])

            # cnT = (ctxT * rstd - mean*rstd) * gT + bT
            xn = wp.tile([P, 2], f32)
            nc.vector.tensor_scalar(
                xn[:],
                stats_in[:, 0:4:2],
                rstd[:],
                mrs[:],
                op0=ALU.mult,
                op1=ALU.subtract,
            )
            tmpg = wp.tile([P, 2], f32)
            nc.vector.tensor_mul(tmpg[:], xn[:], sm[:, 259:261])
            cnT = wp.tile([P, 2], f32)
            nc.vector.tensor_add(cnT[:], tmpg[:], sm[:, 261:263])

            # vT[p, j] = v[j*128+p] = sum_k cn[k] * Wv[k, j*128+p]
            vt_ps = pp.tile([P, 2], f32, tag="vt_ps")
            nc.tensor.matmul(vt_ps[:, 0:1], wv_sb[:, 0:P], cnT[:, 0:1], start=True, stop=False)
            nc.tensor.matmul(vt_ps[:, 0:1], wv_sb[:, C : C + P], cnT[:, 1:2], start=False, stop=True)
            nc.tensor.matmul(vt_ps[:, 1:2], wv_sb[:, P:C], cnT[:, 0:1], start=True, stop=False)
            nc.tensor.matmul(vt_ps[:, 1:2], wv_sb[:, C + P : 2 * C], cnT[:, 1:2], start=False, stop=True)

            # vrep_j[k, m] = v[j*128+k] replicated along free dim (scalar
            # operand read straight from PSUM)
            vrep0 = wp.tile([P, P], f32)
            nc.vector.tensor_scalar_mul(vrep0[:], ones_big[:], vt_ps[:, 0:1])
            vrep1 = wp.tile([P, P], f32)
            nc.vector.tensor_scalar_mul(vrep1[:], ones_big[:], vt_ps[:, 1:2])

            # delta[p, c] = sum_k v[k] Wout[k, c] + bout[c]  (all partitions)
            # bias term first (start=True): it only depends on bout, and PE
            # is otherwise idle in that window.
            delta_ps = pp.tile([P, C], f32, tag="delta_ps")
            nc.tensor.matmul(
                delta_ps[:], mmcast(ones_1[:]), mmcast(bout_sb[:]), start=True, stop=False
            )
            nc.tensor.matmul(
                delta_ps[:], mmcast(vrep0[:]), mmcast(wo_sb[:, 0:C]), start=False, stop=False
            )
            nc.tensor.matmul(
                delta_ps[:], mmcast(vrep1[:]), mmcast(wo_sb[:, C : 2 * C]), start=False, stop=True
            )

            # delta duplicated side by side for [128, 512] adds
            delta2 = wp.tile([P, 2 * C], f32)
            nc.vector.tensor_copy(delta2[:, 0:C], delta_ps[:])
            nc.vector.tensor_copy(delta2[:, C : 2 * C], delta_ps[:])

            # ---- stream: out = img + delta, 4 adds + 4 stores of [128, 512]
            for k in range(4):
                ot = iop.tile([P, 2 * C], f32, tag="img_out", bufs=4, name=f"ot_{k}")
                L, off = k // 2, (k % 2) * 2 * C
                nc.vector.tensor_add(ot[:], imgL[L][:, off : off + 2 * C], delta2[:])
                nc.scalar.dma_start(
                    out.ap()[k * 256 : (k + 1) * 256, :].rearrange(
                        "(j p) c -> p j c", p=P
                    ),
                    ot[:].rearrange("p (j c) -> p j c", j=2),
                )

    nc.compile()
    return nc


def get_nc():
    global _BUILT
    if _BUILT is None:
        _BUILT = _build()
    return _BUILT


def _pack_inputs(inputs):
    f = lambda a: np.ascontiguousarray(np.asarray(a, dtype=np.float32))
    img = f(inputs["img_tokens"])  # [B, N, C]
    param = f(inputs["param_tokens"])  # [B, 16]
    wparam = f(inputs["Wparam"])  # [16, C]
    bparam = f(inputs["bparam"])  # [C]
    gln = f(inputs["ctx_norm_g"])  # [C]
    bln = f(inputs["ctx_norm_b"])  # [C]
    wv = f(np.asarray(inputs["Wkv"], dtype=np.float32)[:, C:])  # [C, C]
    wout = f(inputs["Wout"])  # [C, C]
    bout = f(inputs["bout"])  # [C]

    base = np.zeros((P, SMALLS_F), np.float32)
    base[0:PARAM_DIM, 0:C] = wparam
    for t_i, vec in enumerate((bparam, gln, bln)):
        base[:, 257 + 2 * t_i : 259 + 2 * t_i] = vec.reshape(2, P).T

    in_maps = []
    for b in range(NCORES):
        sm = base.copy()
        sm[0:PARAM_DIM, C] = param[b]
        in_maps.append(
            {
                "img": img[b],
                "smalls": np.ascontiguousarray(sm),
                "wv": wv,
                "wout": wout,
                "bout": bout,
            }
        )
    return in_maps


def kernel(**inputs):
    from concourse.bass_utils import run_bass_kernel_spmd

    nc = get_nc()
    in_maps = _pack_inputs(inputs)
    res = run_bass_kernel_spmd(nc, in_maps, core_ids=list(range(NCORES)))
    out = np.stack([res.results[b]["out"] for b in range(NCORES)], axis=0)
    return out.astype(np.float32)


# revision 19
# speedup vs baseline: 1.3655x; 1.0469x over previous
"""Trainium2 Bass kernel for nn_CrossAttentionBlock_78881369358733.

The reference block's attention is degenerate: every query attends to a
single broadcast context token, so softmax over N identical scores is
exactly uniform and the attention output equals `v` for every position.
The whole module collapses to

    ctx   = param_tokens @ Wparam + bparam          # [B, C]
    v     = layernorm(ctx) @ Wkv[:, C:]             # [B, C]
    delta = v @ Wout + bout                         # [B, C]
    out   = img_tokens + delta[:, None, :]          # [B, N, C]

(q, Wq, img layernorm params, and the k-half of Wkv are dead.)

Sharding: pure data parallel over B — core b handles batch b. Each core
computes its own tiny delta vector on-device (PE matmuls + DVE/ACT ops)
and streams img tiles through a broadcast add.

Perf notes (cost-model-driven):
- each dma_start costs ~625ns on the shared HWDGE ring + ~900ns sem
  propagation, so small tensors are host-packed into one SWDGE-loaded
  array and img moves in few large DMAs;
- LN moments are broadcast to all 128 partitions with a ones*(1/C)
  matmul so every later step uses cheap per-partition scalars;
- the delta matmuls run as float32r (full-rate fp32 PE mode), with
  producers rounding into f32r tiles off the critical path;
- bparam rides inside an augmented K=17 first matmul: [Wparam; bparam]^T
  @ [param; 1].
"""

import sys

if "/opt/trn_rl_repo" not in sys.path:
    sys.path.append("/opt/trn_rl_repo")

import numpy as np

B, N, C = 8, 1024, 256
PARAM_DIM = 16
EPS = 1e-5
P = 128
NCORES = 8
USE_F32R = True

# wpk layout [17, 257]: rows 0:16 = Wparam, row 16 = bparam;
#   col 256 = [param_tokens[b]; 1.0]
# vecs2 layout [128, 4]: cols 0:2 = ctx_norm_g, 2:4 = ctx_norm_b,
#   both as [128, 2] partition layout (x[j*128+p] at [p, j])

_BUILT = None


def _patch_tile_tail(tile_mod):
    """The stock TileContext tail emits a drain with one sem-wait per live
    proc (rejected by this walrus: too many sync waits per TPB_CTRL)
    followed by an EVSEM barrier + sem reset that faults the exec unit on
    this runtime. A single drain is sufficient for one-shot NEFF execution:
    semaphores are re-initialized by each nrt_execute."""
    from bass_rust import ScopedClock

    def _drain_only(self, tick_clock, wait_clock):
        drain_inst = self.nc.sync.drain()
        wait_clock.add_sem_waits(
            drain_inst.ins, ScopedClock({None: tick_clock.global_clock})
        )
        popped = self.nc._tile_sem_poison_stack.pop()
        assert popped is self._sem_poison

    tile_mod.TileContext._drain_and_barrier = _drain_only


def _build():
    import concourse.bacc as bacc
    import concourse.tile as tile
    from concourse import mybir

    _patch_tile_tail(tile)

    f32 = mybir.dt.float32
    f32r = mybir.dt.float32r if USE_F32R else mybir.dt.float32
    AF = mybir.ActivationFunctionType
    ALU = mybir.AluOpType

    nc = bacc.Bacc("TRN2", target_bir_lowering=False, debug=False)

    img = nc.dram_tensor("img", [N, C], f32, kind="ExternalInput")
    wpk = nc.dram_tensor("wpk", [PARAM_DIM + 1, C + 1], f32, kind="ExternalInput")
    vecs2 = nc.dram_tensor("vecs2", [P, 4], f32, kind="ExternalInput")
    wv = nc.dram_tensor("wv", [C, C], f32, kind="ExternalInput")
    wout = nc.dram_tensor("wout", [C, C], f32, kind="ExternalInput")
    bout = nc.dram_tensor("bout", [C], f32, kind="ExternalInput")
    out = nc.dram_tensor("out", [N, C], f32, kind="ExternalOutput")

    with tile.TileContext(nc) as tc:
        with (
            tc.tile_pool(name="w", bufs=1) as wp,
            tc.tile_pool(name="io", bufs=1) as iop,
            tc.tile_pool(name="ps", bufs=1, space="PSUM") as pp,
        ):
            # ---- SP ring loads in chain-dependency order; the tiny
            # chain-critical wpk goes first ----
            wpk_sb = wp.tile([PARAM_DIM + 1, C + 1], f32)
            nc.sync.dma_start(wpk_sb[:], wpk.ap())
            bout_sb = wp.tile([1, C], f32)
            nc.sync.dma_start(bout_sb[:], bout.ap().rearrange("(o c) -> o c", o=1))
            # wv_sb[p, j*C + c] = Wv[j*128 + p, c] (one DMA per matrix)
            wv_sb = wp.tile([P, 2 * C], f32)
            nc.sync.dma_start(
                wv_sb[:].rearrange("p (j c) -> p j c", j=2),
                wv.ap().rearrange("(j p) c -> p j c", p=P),
            )
            wo_sb = wp.tile([P, 2 * C], f32)
            nc.sync.dma_start(
                wo_sb[:].rearrange("p (j c) -> p j c", j=2),
                wout.ap().rearrange("(j p) c -> p j c", p=P),
            )
            # gamma/beta via SWDGE (Pool) — off the HWDGE ring; needed late
            vecs2_sb = wp.tile([P, 4], f32)
            nc.gpsimd.dma_start(vecs2_sb[:], vecs2.ap())
            # img: 2 DMAs of 512KB; imgL[L][p, j*C+c] = img[L*512 + j*128 + p, c]
            imgL = []
            for L in range(2):
                t = iop.tile([P, 4 * C], f32, tag=f"img_in{L}", name=f"img_in_{L}")
                nc.sync.dma_start(
                    t[:].rearrange("p (j c) -> p j c", j=4),
                    img.ap()[L * 512 : (L + 1) * 512, :].rearrange(
                        "(j p) c -> p j c", p=P
                    ),
                )
                imgL.append(t)

            # ---- constants (DVE memsets; off the critical path) ----
            ones_1 = wp.tile([1, P], f32)
            nc.vector.memset(ones_1[:], 1.0)
            ones_1r = wp.tile([1, P], f32r)
            nc.vector.tensor_copy(ones_1r[:], ones_1[:])
            ones_big = wp.tile([P, P], f32)
            nc.vector.memset(ones_big[:], 1.0)
            invC_mat = wp.tile([P, P], f32)
            nc.vector.memset(invC_mat[:], 1.0 / C)
            eps_col = wp.tile([P, 1], f32)
            nc.vector.memset(eps_col[:], EPS)

            # f32r-rounded copies of the delta-matmul operands (DVE, run as
            # soon as their DMAs land; off the chain's critical path)
            bout_r = wp.tile([1, C], f32r)
            nc.vector.tensor_copy(bout_r[:], bout_sb[:])

            # ---- delta chain (partition layout: x[j*128+p] at [p, j]) ----
            # K=17 augmented matmul: ctxT + bparamT directly
            KA = PARAM_DIM + 1
            ctx_ps = pp.tile([P, 2], f32, tag="ctx_ps")
            nc.tensor.matmul(
                ctx_ps[:, 0:1], wpk_sb[0:KA, 0:P], wpk_sb[0:KA, C : C + 1],
                start=True, stop=True,
            )
            nc.tensor.matmul(
                ctx_ps[:, 1:2], wpk_sb[0:KA, P:C], wpk_sb[0:KA, C : C + 1],
                start=True, stop=True,
            )

            # stats_in interleaved: cols (x0, x^2_0, x1, x^2_1)
            stats_in = wp.tile([P, 4], f32)
            nc.vector.tensor_copy(stats_in[:, 0:4:2], ctx_ps[:])
            nc.vector.tensor_mul(
                stats_in[:, 1:4:2], stats_in[:, 0:4:2], stats_in[:, 0:4:2]
            )

            # moments broadcast to ALL partitions: ones(1/C)^T @ stats
            sums_ps = pp.tile([P, 4], f32, tag="sums_ps")
            nc.tensor.matmul(sums_ps[:], invC_mat[:], stats_in[:], start=True, stop=True)
            sums_sb = wp.tile([P, 4], f32)
            nc.vector.tensor_copy(sums_sb[:], sums_ps[:])

            moms = wp.tile([P, 2], f32)  # (mean, E[x^2]) on every partition
            nc.vector.tensor_add(moms[:], sums_sb[:, 0:2], sums_sb[:, 2:4])
            m2 = wp.tile([P, 1], f32)
            nc.vector.tensor_mul(m2[:], moms[:, 0:1], moms[:, 0:1])
            var = wp.tile([P, 1], f32)
            nc.vector.tensor_sub(var[:], moms[:, 1:2], m2[:])
            # wo_r rounding rides the DVE idle gap while ACT runs Sqrt
            wo_r = wp.tile([P, 2 * C], f32r)
            nc.vector.tensor_copy(wo_r[:], wo_sb[:])
            sd = wp.tile([P, 1], f32)
            nc.scalar.activation(sd[:], var[:], AF.Sqrt, bias=eps_col[:])
            rstd = wp.tile([P, 1], f32)
            nc.vector.reciprocal(rstd[:], sd[:])
            mrs = wp.tile([P, 1], f32)
            nc.vector.tensor_mul(mrs[:], moms[:, 0:1], rstd[:])

            # cnT = (ctxT * rstd - mean*rstd) * gT + bT
            xn = wp.tile([P, 2], f32)
            nc.vector.tensor_scalar(
                xn[:],
                stats_in[:, 0:4:2],
                rstd[:],
                mrs[:],
                op0=ALU.mult,
                op1=ALU.subtract,
            )
            tmpg = wp.tile([P, 2], f32)
            nc.vector.tensor_mul(tmpg[:], xn[:], vecs2_sb[:, 0:2])
            cnT = wp.tile([P, 2], f32)
            nc.vector.tensor_add(cnT[:], tmpg[:], vecs2_sb[:, 2:4])

            # vT[p, j] = v[j*128+p] = sum_k cn[k] * Wv[k, j*128+p]
            vt_ps = pp.tile([P, 2], f32, tag="vt_ps")
            nc.tensor.matmul(vt_ps[:, 0:1], wv_sb[:, 0:P], cnT[:, 0:1], start=True, stop=False)
            nc.tensor.matmul(vt_ps[:, 0:1], wv_sb[:, C : C + P], cnT[:, 1:2], start=False, stop=True)
            nc.tensor.matmul(vt_ps[:, 1:2], wv_sb[:, P:C], cnT[:, 0:1], start=True, stop=False)
            nc.tensor.matmul(vt_ps[:, 1:2], wv_sb[:, C + P : 2 * C], cnT[:, 1:2], start=False, stop=True)

            # vrep_j[k, m] = v[j*128+k] replicated along free dim (scalar
            # operand read straight from PSUM), rounded to f32r on write
            vrep0 = wp.tile([P, P], f32r)
            nc.vector.tensor_scalar_mul(vrep0[:], ones_big[:], vt_ps[:, 0:1])
            vrep1 = wp.tile([P, P], f32r)
            nc.vector.tensor_scalar_mul(vrep1[:], ones_big[:], vt_ps[:, 1:2])

            # delta[p, c] = sum_k v[k] Wout[k, c] + bout[c]  (all partitions)
            # bias term first (start=True): it only depends on bout, and PE
            # is otherwise idle in that window.
            delta_ps = pp.tile([P, C], f32, tag="delta_ps")
            nc.tensor.matmul(
                delta_ps[:], ones_1r[:], bout_r[:], start=True, stop=False
            )
            nc.tensor.matmul(
                delta_ps[:], vrep0[:], wo_r[:, 0:C], start=False, stop=False
            )
            nc.tensor.matmul(
                delta_ps[:], vrep1[:], wo_r[:, C : 2 * C], start=False, stop=True
            )

            # delta duplicated side by side for [128, 512] adds
            delta2 = wp.tile([P, 2 * C], f32)
            nc.vector.tensor_copy(delta2[:, 0:C], delta_ps[:])
            nc.vector.tensor_copy(delta2[:, C : 2 * C], delta_ps[:])

            # ---- stream: out = img + delta, 4 adds + 4 stores of [128, 512]
            for k in range(4):
                ot = iop.tile([P, 2 * C], f32, tag="img_out", bufs=4, name=f"ot_{k}")
                L, off = k // 2, (k % 2) * 2 * C
                nc.vector.tensor_add(ot[:], imgL[L][:, off : off + 2 * C], delta2[:])
                nc.scalar.dma_start(
                    out.ap()[k * 256 : (k + 1) * 256, :].rearrange(
                        "(j p) c -> p j c", p=P
                    ),
                    ot[:].rearrange("p (j c) -> p j c", j=2),
                )

    nc.compile()
    return nc


def get_nc():
    global _BUILT
    if _BUILT is None:
        _BUILT = _build()
    return _BUILT


def _pack_inputs(inputs):
    f = lambda a: np.ascontiguousarray(np.asarray(a, dtype=np.float32))
    img = f(inputs["img_tokens"])  # [B, N, C]
    param = f(inputs["param_tokens"])  # [B, 16]
    wparam = f(inputs["Wparam"])  # [16, C]
    bparam = f(inputs["bparam"])  # [C]
    gln = f(inputs["ctx_norm_g"])  # [C]
    bln = f(inputs["ctx_norm_b"])  # [C]
    wv = f(np.asarray(inputs["Wkv"], dtype=np.float32)[:, C:])  # [C, C]
    wout = f(inputs["Wout"])  # [C, C]
    bout = f(inputs["bout"])  # [C]

    base = np.zeros((PARAM_DIM + 1, C + 1), np.float32)
    base[0:PARAM_DIM, 0:C] = wparam
    base[PARAM_DIM, 0:C] = bparam
    base[PARAM_DIM, C] = 1.0

    vecs2 = np.empty((P, 4), np.float32)
    vecs2[:, 0:2] = gln.reshape(2, P).T
    vecs2[:, 2:4] = bln.reshape(2, P).T
    vecs2 = np.ascontiguousarray(vecs2)

    in_maps = []
    for b in range(NCORES):
        wpk = base.copy()
        wpk[0:PARAM_DIM, C] = param[b]
        in_maps.append(
            {
                "img": img[b],
                "wpk": np.ascontiguousarray(wpk),
                "vecs2": vecs2,
                "wv": wv,
                "wout": wout,
                "bout": bout,
            }
        )
    return in_maps


def kernel(**inputs):
    from concourse.bass_utils import run_bass_kernel_spmd

    nc = get_nc()
    in_maps = _pack_inputs(inputs)
    res = run_bass_kernel_spmd(nc, in_maps, core_ids=list(range(NCORES)))
    out = np.stack([res.results[b]["out"] for b in range(NCORES)], axis=0)
    return out.astype(np.float32)


# revision 20
# speedup vs baseline: 1.4565x; 1.0666x over previous
"""Trainium2 Bass kernel for nn_CrossAttentionBlock_78881369358733.

The reference block's attention is degenerate: every query attends to a
single broadcast context token, so softmax over N identical scores is
exactly uniform and the attention output equals `v` for every position.
The whole module collapses to

    ctx   = param_tokens @ Wparam + bparam          # [B, C]
    v     = layernorm(ctx) @ Wkv[:, C:]             # [B, C]
    delta = v @ Wout + bout                         # [B, C]
    out   = img_tokens + delta[:, None, :]          # [B, N, C]

(q, Wq, img layernorm params, and the k-half of Wkv are dead.)

Sharding: pure data parallel over B — core b handles batch b. Each core
computes its own tiny delta vector on-device (PE matmuls + DVE/ACT ops)
and streams img tiles through a broadcast add.

Perf notes (cost-model-driven):
- each dma_start costs ~625ns on the shared HWDGE ring + ~900ns sem
  propagation, so small tensors are host-packed into one SWDGE-loaded
  array and img moves in few large DMAs;
- LN moments are broadcast to all 128 partitions with a ones*(1/C)
  matmul so every later step uses cheap per-partition scalars;
- the delta matmuls run as float32r (full-rate fp32 PE mode), with
  producers rounding into f32r tiles off the critical path;
- bparam rides inside an augmented K=17 first matmul: [Wparam; bparam]^T
  @ [param; 1].
"""

import sys

if "/opt/trn_rl_repo" not in sys.path:
    sys.path.append("/opt/trn_rl_repo")

import numpy as np

B, N, C = 8, 1024, 256
PARAM_DIM = 16
EPS = 1e-5
P = 128
NCORES = 8
USE_F32R = False

# wpk layout [17, 257]: rows 0:16 = Wparam, row 16 = bparam;
#   col 256 = [param_tokens[b]; 1.0]
# vecs2 layout [128, 4]: cols 0:2 = ctx_norm_g, 2:4 = ctx_norm_b,
#   both as [128, 2] partition layout (x[j*128+p] at [p, j])

_BUILT = None


def _patch_tile_tail(tile_mod):
    """The stock TileContext tail emits a drain with one sem-wait per live
    proc (rejected by this walrus: too many sync waits per TPB_CTRL)
    followed by an EVSEM barrier + sem reset that faults the exec unit on
    this runtime. A single drain is sufficient for one-shot NEFF execution:
    semaphores are re-initialized by each nrt_execute."""
    from bass_rust import ScopedClock

    def _drain_only(self, tick_clock, wait_clock):
        drain_inst = self.nc.sync.drain()
        wait_clock.add_sem_waits(
            drain_inst.ins, ScopedClock({None: tick_clock.global_clock})
        )
        popped = self.nc._tile_sem_poison_stack.pop()
        assert popped is self._sem_poison

    tile_mod.TileContext._drain_and_barrier = _drain_only


def _build():
    import concourse.bacc as bacc
    import concourse.tile as tile
    from concourse import mybir

    _patch_tile_tail(tile)

    f32 = mybir.dt.float32
    f32r = mybir.dt.float32r if USE_F32R else mybir.dt.float32
    AF = mybir.ActivationFunctionType
    ALU = mybir.AluOpType

    nc = bacc.Bacc("TRN2", target_bir_lowering=False, debug=False)

    img = nc.dram_tensor("img", [N, C], f32, kind="ExternalInput")
    wpk = nc.dram_tensor("wpk", [PARAM_DIM + 1, C + 1], f32, kind="ExternalInput")
    vecs2 = nc.dram_tensor("vecs2", [P, 4], f32, kind="ExternalInput")
    wv = nc.dram_tensor("wv", [C, C], f32, kind="ExternalInput")
    wout = nc.dram_tensor("wout", [C, C], f32, kind="ExternalInput")
    bout = nc.dram_tensor("bout", [C], f32, kind="ExternalInput")
    out = nc.dram_tensor("out", [N, C], f32, kind="ExternalOutput")

    with tile.TileContext(nc) as tc:
        with (
            tc.tile_pool(name="w", bufs=1) as wp,
            tc.tile_pool(name="io", bufs=1) as iop,
            tc.tile_pool(name="ps", bufs=1, space="PSUM") as pp,
        ):
            # ---- SP ring loads in chain-dependency order; the tiny
            # chain-critical wpk goes first ----
            wpk_sb = wp.tile([PARAM_DIM + 1, C + 1], f32)
            nc.sync.dma_start(wpk_sb[:], wpk.ap())
            bout_sb = wp.tile([1, C], f32)
            nc.sync.dma_start(bout_sb[:], bout.ap().rearrange("(o c) -> o c", o=1))
            # wv_sb[p, j*C + c] = Wv[j*128 + p, c] (one DMA per matrix)
            wv_sb = wp.tile([P, 2 * C], f32)
            nc.sync.dma_start(
                wv_sb[:].rearrange("p (j c) -> p j c", j=2),
                wv.ap().rearrange("(j p) c -> p j c", p=P),
            )
            wo_sb = wp.tile([P, 2 * C], f32)
            nc.sync.dma_start(
                wo_sb[:].rearrange("p (j c) -> p j c", j=2),
                wout.ap().rearrange("(j p) c -> p j c", p=P),
            )
            # gamma/beta via SWDGE (Pool) — off the HWDGE ring; needed late
            vecs2_sb = wp.tile([P, 4], f32)
            nc.gpsimd.dma_start(vecs2_sb[:], vecs2.ap())
            # img: 2 DMAs of 512KB; imgL[L][p, j*C+c] = img[L*512 + j*128 + p, c]
            imgL = []
            for L in range(2):
                t = iop.tile([P, 4 * C], f32, tag=f"img_in{L}", name=f"img_in_{L}")
                nc.sync.dma_start(
                    t[:].rearrange("p (j c) -> p j c", j=4),
                    img.ap()[L * 512 : (L + 1) * 512, :].rearrange(
                        "(j p) c -> p j c", p=P
                    ),
                )
                imgL.append(t)

            # ---- constants (DVE memsets; off the critical path) ----
            ones_1 = wp.tile([1, P], f32)
            nc.vector.memset(ones_1[:], 1.0)
            if USE_F32R:
                ones_1r = wp.tile([1, P], f32r)
                nc.vector.tensor_copy(ones_1r[:], ones_1[:])
            else:
                ones_1r = ones_1
            ones_big = wp.tile([P, P], f32)
            nc.vector.memset(ones_big[:], 1.0)
            invC_mat = wp.tile([P, P], f32)
            nc.vector.memset(invC_mat[:], 1.0 / C)
            eps_col = wp.tile([P, 1], f32)
            nc.vector.memset(eps_col[:], EPS)

            # f32r-rounded copies of the delta-matmul operands (DVE, run as
            # soon as their DMAs land; off the chain's critical path)
            if USE_F32R:
                bout_r = wp.tile([1, C], f32r)
                nc.vector.tensor_copy(bout_r[:], bout_sb[:])
            else:
                bout_r = bout_sb

            # ---- delta chain (partition layout: x[j*128+p] at [p, j]) ----
            # K=17 augmented matmul: ctxT + bparamT directly
            KA = PARAM_DIM + 1
            ctx_ps = pp.tile([P, 2], f32, tag="ctx_ps")
            nc.tensor.matmul(
                ctx_ps[:, 0:1], wpk_sb[0:KA, 0:P], wpk_sb[0:KA, C : C + 1],
                start=True, stop=True,
            )
            nc.tensor.matmul(
                ctx_ps[:, 1:2], wpk_sb[0:KA, P:C], wpk_sb[0:KA, C : C + 1],
                start=True, stop=True,
            )

            # stats_in interleaved: cols (x0, x^2_0, x1, x^2_1)
            stats_in = wp.tile([P, 4], f32)
            nc.vector.tensor_copy(stats_in[:, 0:4:2], ctx_ps[:])
            nc.vector.tensor_mul(
                stats_in[:, 1:4:2], stats_in[:, 0:4:2], stats_in[:, 0:4:2]
            )

            # moments broadcast to ALL partitions: ones(1/C)^T @ stats
            sums_ps = pp.tile([P, 4], f32, tag="sums_ps")
            nc.tensor.matmul(sums_ps[:], invC_mat[:], stats_in[:], start=True, stop=True)
            sums_sb = wp.tile([P, 4], f32)
            nc.vector.tensor_copy(sums_sb[:], sums_ps[:])

            moms = wp.tile([P, 2], f32)  # (mean, E[x^2]) on every partition
            nc.vector.tensor_add(moms[:], sums_sb[:, 0:2], sums_sb[:, 2:4])
            m2 = wp.tile([P, 1], f32)
            nc.vector.tensor_mul(m2[:], moms[:, 0:1], moms[:, 0:1])
            var = wp.tile([P, 1], f32)
            nc.vector.tensor_sub(var[:], moms[:, 1:2], m2[:])
            if USE_F32R:
                # wo_r rounding rides the DVE idle gap while ACT runs Sqrt
                wo_r = wp.tile([P, 2 * C], f32r)
                nc.vector.tensor_copy(wo_r[:], wo_sb[:])
            else:
                wo_r = wo_sb
            sd = wp.tile([P, 1], f32)
            nc.scalar.activation(sd[:], var[:], AF.Sqrt, bias=eps_col[:])
            rstd = wp.tile([P, 1], f32)
            nc.vector.reciprocal(rstd[:], sd[:])
            mrs = wp.tile([P, 1], f32)
            nc.vector.tensor_mul(mrs[:], moms[:, 0:1], rstd[:])

            # cnT = (ctxT * rstd - mean*rstd) * gT + bT
            xn = wp.tile([P, 2], f32)
            nc.vector.tensor_scalar(
                xn[:],
                stats_in[:, 0:4:2],
                rstd[:],
                mrs[:],
                op0=ALU.mult,
                op1=ALU.subtract,
            )
            tmpg = wp.tile([P, 2], f32)
            nc.vector.tensor_mul(tmpg[:], xn[:], vecs2_sb[:, 0:2])
            cnT = wp.tile([P, 2], f32)
            nc.vector.tensor_add(cnT[:], tmpg[:], vecs2_sb[:, 2:4])

            # vT[p, j] = v[j*128+p] = sum_k cn[k] * Wv[k, j*128+p]
            vt_ps = pp.tile([P, 2], f32, tag="vt_ps")
            nc.tensor.matmul(vt_ps[:, 0:1], wv_sb[:, 0:P], cnT[:, 0:1], start=True, stop=False)
            nc.tensor.matmul(vt_ps[:, 0:1], wv_sb[:, C : C + P], cnT[:, 1:2], start=False, stop=True)
            nc.tensor.matmul(vt_ps[:, 1:2], wv_sb[:, P:C], cnT[:, 0:1], start=True, stop=False)
            nc.tensor.matmul(vt_ps[:, 1:2], wv_sb[:, C + P : 2 * C], cnT[:, 1:2], start=False, stop=True)

            # vrep_j[k, m] = v[j*128+k] replicated along free dim (scalar
            # operand read straight from PSUM), rounded to f32r on write
            vrep0 = wp.tile([P, P], f32r)
            nc.vector.tensor_scalar_mul(vrep0[:], ones_big[:], vt_ps[:, 0:1])
            vrep1 = wp.tile([P, P], f32r)
            nc.vector.tensor_scalar_mul(vrep1[:], ones_big[:], vt_ps[:, 1:2])

            # delta[p, c] = sum_k v[k] Wout[k, c] + bout[c]  (all partitions)
            # bias term first (start=True): it only depends on bout, and PE
            # is otherwise idle in that window.
            delta_ps = pp.tile([P, C], f32, tag="delta_ps")
            nc.tensor.matmul(
                delta_ps[:], ones_1r[:], bout_r[:], start=True, stop=False
            )
            nc.tensor.matmul(
                delta_ps[:], vrep0[:], wo_r[:, 0:C], start=False, stop=False
            )
            nc.tensor.matmul(
                delta_ps[:], vrep1[:], wo_r[:, C : 2 * C], start=False, stop=True
            )

            # delta duplicated side by side for [128, 512] adds
            delta2 = wp.tile([P, 2 * C], f32)
            nc.vector.tensor_copy(delta2[:, 0:C], delta_ps[:])
            nc.vector.tensor_copy(delta2[:, C : 2 * C], delta_ps[:])

            # ---- stream: out = img + delta, 4 adds + 4 stores of [128, 512]
            for k in range(4):
                ot = iop.tile([P, 2 * C], f32, tag="img_out", bufs=4, name=f"ot_{k}")
                L, off = k // 2, (k % 2) * 2 * C
                nc.vector.tensor_add(ot[:], imgL[L][:, off : off + 2 * C], delta2[:])
                nc.scalar.dma_start(
                    out.ap()[k * 256 : (k + 1) * 256, :].rearrange(
                        "(j p) c -> p j c", p=P
                    ),
                    ot[:].rearrange("p (j c) -> p j c", j=2),
                )

    nc.compile()
    return nc


def get_nc():
    global _BUILT
    if _BUILT is None:
        _BUILT = _build()
    return _BUILT


def _pack_inputs(inputs):
    f = lambda a: np.ascontiguousarray(np.asarray(a, dtype=np.float32))
    img = f(inputs["img_tokens"])  # [B, N, C]
    param = f(inputs["param_tokens"])  # [B, 16]
    wparam = f(inputs["Wparam"])  # [16, C]
    bparam = f(inputs["bparam"])  # [C]
    gln = f(inputs["ctx_norm_g"])  # [C]
    bln = f(inputs["ctx_norm_b"])  # [C]
    wv = f(np.asarray(inputs["Wkv"], dtype=np.float32)[:, C:])  # [C, C]
    wout = f(inputs["Wout"])  # [C, C]
    bout = f(inputs["bout"])  # [C]

    base = np.zeros((PARAM_DIM + 1, C + 1), np.float32)
    base[0:PARAM_DIM, 0:C] = wparam
    base[PARAM_DIM, 0:C] = bparam
    base[PARAM_DIM, C] = 1.0

    vecs2 = np.empty((P, 4), np.float32)
    vecs2[:, 0:2] = gln.reshape(2, P).T
    vecs2[:, 2:4] = bln.reshape(2, P).T
    vecs2 = np.ascontiguousarray(vecs2)

    in_maps = []
    for b in range(NCORES):
        wpk = base.copy()
        wpk[0:PARAM_DIM, C] = param[b]
        in_maps.append(
            {
                "img": img[b],
                "wpk": np.ascontiguousarray(wpk),
                "vecs2": vecs2,
                "wv": wv,
                "wout": wout,
                "bout": bout,
            }
        )
    return in_maps


def kernel(**inputs):
    from concourse.bass_utils import run_bass_kernel_spmd

    nc = get_nc()
    in_maps = _pack_inputs(inputs)
    res = run_bass_kernel_spmd(nc, in_maps, core_ids=list(range(NCORES)))
    out = np.stack([res.results[b]["out"] for b in range(NCORES)], axis=0)
    return out.astype(np.float32)
